# revision 19
# baseline (speedup 1.0000x reference)
"""Trainium2 Bass kernel: ragged pathway-transformer (nn_DOIT_39436389712066).

Sharding over 8 NeuronCores:
  Phase A (pathway/expert parallel): core c owns pathways [25c,25c+25).
    Host pre-gathers x rows per pathway into transposed lhsT layout
    (bf16, K padded 1200->1280). Device grouped-GEMM produces pathway
    tokens for all 256 batches -> AllToAll to batch-parallel layout.
    nc-token GEMM sharded over K (5632 rows/core) -> ReduceScatter.
  Phase B (batch parallel): core c owns batches [32c,32c+32).
    4-layer transformer, residual kept transposed (hT [2x128 dim chunks,
    tokens]); LN stats via column-select matmuls; bf16 matmuls with fp32
    PSUM; per-(batch,head) attention with transposed scores; softmax
    denominators via indicator matmuls, reciprocal = exp(-ln(x));
    exact-erf gelu on ScalarE. CLS head on-device, [20,32] per core.

kernel(**inputs) hardcodes all shapes, compiles once per process, runs
on cores 0-7 via run_bass_kernel_spmd, reassembles the [256,20] output.
"""

import numpy as np
import ml_dtypes

import concourse.bass as bass
import concourse.mybir as mybir
import concourse.tile as tile
from concourse.bass import ds, ts
from concourse.bass_utils import run_bass_kernel_spmd

F32 = mybir.dt.float32
BF16 = mybir.dt.bfloat16
AF = mybir.ActivationFunctionType
F32R = mybir.dt.float32r

B, G, O, P, MAXG = 256, 20000, 3, 200, 400
NCTOK = 15000
DIM, DEPTH, HEADS, DH, MLP, OUT = 256, 4, 8, 64, 1024, 20
INNER = HEADS * DH
T = P + 2
EPS = 1e-5
NCORES = 8
PL = P // NCORES
BL = B // NCORES
NPAIR = BL // 2
KPATHP = 1280  # 10*128 (zero padded from 1200)
KNC = NCTOK * O // NCORES  # 5625
KNCP = 5632  # 44*128
TCH = [(0, 128), (128, 74)]

_CACHE = {}


# --------------------------------------------------------------------------
# device program
# --------------------------------------------------------------------------
def _build():
    nc = bass.Bass(
        "TRN2",
        target_bir_lowering=False,
        debug=False,
        enable_asserts=False,
        num_devices=NCORES,
    )

    def din(name, shape, dt=BF16):
        return nc.dram_tensor(name, shape, dt, kind="ExternalInput").ap()

    gT = din("gT", [PL, KPATHP, B])
    wp = din("wp", [PL, KPATHP, DIM])
    bp = din("bp", [1, PL * DIM])
    ncg = din("ncg", [KNCP, B])
    wnc = din("wnc", [KNCP, DIM])
    bnc = din("bnc", [1, DIM])
    cls32 = din("cls32", [1, DIM], F32)
    wqkv = din("wqkv", [128, DEPTH * 2 * 3 * INNER])
    wo = din("wo", [128, DEPTH * 4 * DIM])
    wf1 = din("wf1", [128, DEPTH * 2 * MLP])
    wf2 = din("wf2", [128, DEPTH * 8 * DIM])
    bo16 = din("bo16", [1, DEPTH * DIM])
    bf1_16 = din("bf1_16", [1, DEPTH * MLP])
    bf2_16 = din("bf2_16", [1, DEPTH * DIM])
    lnw = din("lnw", [128, 36], F32)
    lnhrow = din("lnhrow", [2, DIM])
    colsel = din("colsel", [128, 127])
    colsel32 = din("colsel32", [128, 127], F32R)
    ones1 = din("ones1", [1, 128])
    onesn = din("onesn", [1, 404])
    esel = din("esel", [8, 4 * 128])
    ident = din("ident", [128, 128], F32)
    identb = din("identb", [128, 128])
    wh1 = din("wh1", [128, 2 * (4 * DIM)])
    wh2 = din("wh2", [128, 8 * OUT])
    bh1 = din("bh1", [1, 4 * DIM])
    bh2c = din("bh2c", [OUT, 1], F32)
    out_d = nc.dram_tensor("out", [OUT, BL], F32, kind="ExternalOutput").ap()

    tcx = tile.TileContext(nc)
    tc = tcx.__enter__()

    singles = tc.alloc_tile_pool(name="singles", bufs=1)
    dram = tc.alloc_tile_pool(name="dram", bufs=1, space="DRAM")

    colsel_t = singles.tile([128, 127], BF16)
    nc.sync.dma_start(out=colsel_t, in_=colsel)
    colsel32_t = singles.tile([128, 127], F32R)
    nc.sync.dma_start(out=colsel32_t, in_=colsel32)
    ones1_t = singles.tile([1, 128], BF16)
    nc.sync.dma_start(out=ones1_t, in_=ones1)
    onesn_t = singles.tile([1, 404], BF16)
    nc.sync.dma_start(out=onesn_t, in_=onesn)
    esel_t = singles.tile([8, 4, 128], BF16)
    nc.sync.dma_start(out=esel_t, in_=esel.rearrange("p (c m) -> p c m", c=4))
    ident_t = singles.tile([128, 128], F32)
    nc.sync.dma_start(out=ident_t, in_=ident)
    identb_t = singles.tile([128, 128], BF16)
    nc.sync.dma_start(out=identb_t, in_=identb)
    lnw_t = singles.tile([128, 36], F32)
    nc.sync.dma_start(out=lnw_t, in_=lnw)
    lnhrow_t = singles.tile([2, DIM], BF16)
    nc.sync.dma_start(out=lnhrow_t, in_=lnhrow)
    bo_t = singles.tile([1, DEPTH * DIM], BF16)
    nc.sync.dma_start(out=bo_t, in_=bo16)
    bf1_t = singles.tile([1, DEPTH * MLP], BF16)
    nc.sync.dma_start(out=bf1_t, in_=bf1_16)
    bf2_t = singles.tile([1, DEPTH * DIM], BF16)
    nc.sync.dma_start(out=bf2_t, in_=bf2_16)
    wh1_t = singles.tile([128, 2, 4 * DIM], BF16)
    nc.sync.dma_start(out=wh1_t, in_=wh1.rearrange("p (c f) -> p c f", c=2))
    wh2_t = singles.tile([128, 8, OUT], BF16)
    nc.sync.dma_start(out=wh2_t, in_=wh2.rearrange("p (k f) -> p k f", k=8))
    bh1_t = singles.tile([1, 4 * DIM], BF16)
    nc.sync.dma_start(out=bh1_t, in_=bh1)
    bh2_t = singles.tile([OUT, 1], F32)
    nc.sync.dma_start(out=bh2_t, in_=bh2c)
    cls_t = singles.tile([1, DIM], F32)
    nc.sync.dma_start(out=cls_t, in_=cls32)
    bp_t = singles.tile([1, PL, DIM], BF16)
    nc.sync.dma_start(out=bp_t, in_=bp.rearrange("o (p d) -> o p d", p=PL))
    bnc_t = singles.tile([1, DIM], BF16)
    nc.sync.dma_start(out=bnc_t, in_=bnc)

    a2a_in = dram.tile([NCORES, PL, BL, DIM], F32)
    a2a_out = dram.tile([NCORES, PL, BL, DIM], F32)
    ncp_in = dram.tile([B, DIM], F32)
    ncp_out = dram.tile([BL, DIM], F32)

    hT = [singles.tile([128, 2, 404], F32R, name=f"hT{i}") for i in range(NPAIR)]

    # ================= PHASE A =================
    with (
        tc.tile_pool(name="pa_in", bufs=3) as pa_in,
        tc.tile_pool(name="pa_st", bufs=3) as pa_st,
        tc.tile_pool(name="pa_ps", bufs=3, space="PSUM") as pa_ps,
        tc.tile_pool(name="pa_ps2", bufs=1, space="PSUM") as pa_ps2,
    ):
        for p in range(PL):
            g_t = pa_in.tile([128, 10, B], BF16, name="g")
            nc.sync.dma_start(out=g_t, in_=gT[p].rearrange("(a q) b -> q a b", q=128))
            w_t = pa_in.tile([128, 10, DIM], BF16, name="w")
            nc.sync.dma_start(out=w_t, in_=wp[p].rearrange("(a q) d -> q a d", q=128))
            for bc in range(2):
                ps = pa_ps.tile([128, DIM], F32, name="ps")
                for a in range(10):
                    nc.tensor.matmul(ps, g_t[:, a, ts(bc, 128)], w_t[:, a, :],
                                     start=(a == 0), stop=False, skip_group_check=True)
                nc.tensor.matmul(ps, ones1_t, bp_t[0:1, p, :],
                                 start=False, stop=True, skip_group_check=True)
                st = pa_st.tile([128, DIM], F32, name="st")
                nc.vector.tensor_copy(st, ps)
                for j in range(4):
                    nc.sync.dma_start(
                        out=a2a_in[4 * bc + j, p], in_=st[ts(j, 32), :])
        ncps = [pa_ps2.tile([128, DIM], F32, name=f"ncps{bc}") for bc in range(2)]
        for kg in range(4):
            ncg_t = pa_in.tile([128, 11, B], BF16, name="ncg")
            nc.sync.dma_start(
                out=ncg_t,
                in_=ncg.rearrange("(g a q) b -> g q a b", g=4, q=128)[kg])
            wnc_t = pa_in.tile([128, 11, DIM], BF16, name="wnc")
            nc.sync.dma_start(
                out=wnc_t,
                in_=wnc.rearrange("(g a q) d -> g q a d", g=4, q=128)[kg])
            for bc in range(2):
                for a in range(11):
                    nc.tensor.matmul(ncps[bc], ncg_t[:, a, ts(bc, 128)],
                                     wnc_t[:, a, :],
                                     start=(kg == 0 and a == 0), stop=False,
                                     skip_group_check=True)
        for bc in range(2):
            nc.tensor.matmul(ncps[bc], ones1_t, bnc_t, start=False, stop=True,
                             skip_group_check=True)
            st = pa_st.tile([128, DIM], F32, name="st")
            nc.vector.tensor_copy(st, ncps[bc])
            nc.sync.dma_start(out=ncp_in[ts(bc, 128), :], in_=st)

    nc.gpsimd.collective_compute(
        "AllToAll", mybir.AluOpType.bypass,
        replica_groups=[list(range(NCORES))],
        ins=[a2a_in.opt()], outs=[a2a_out.opt()])
    nc.gpsimd.collective_compute(
        "ReduceScatter", mybir.AluOpType.add,
        replica_groups=[list(range(NCORES))],
        ins=[ncp_in.opt()], outs=[ncp_out.opt()])

    # ================= PHASE B0: assemble hT =================
    with (
        tc.tile_pool(name="tm", bufs=4) as tmp_pool,
        tc.tile_pool(name="b0ps", bufs=3, space="PSUM") as b0ps,
    ):
        for b in range(BL):
            pair, half = b // 2, b % 2
            for tci, (t0, tn) in enumerate(TCH):
                tm = tmp_pool.tile([128, DIM], F32, name="tm")
                if tci == 0:
                    nc.sync.dma_start(out=tm[0:1, :], in_=cls_t)
                    nc.sync.dma_start(
                        out=tm[1:101, :],
                        in_=a2a_out[0:4, :, b].rearrange("c p d -> (c p) d"))
                    nc.sync.dma_start(out=tm[101:126, :], in_=a2a_out[4, :, b])
                    nc.sync.dma_start(out=tm[126:128, :], in_=a2a_out[5, 0:2, b])
                else:
                    nc.sync.dma_start(out=tm[0:23, :], in_=a2a_out[5, 2:25, b])
                    nc.sync.dma_start(
                        out=tm[23:73, :],
                        in_=a2a_out[6:8, :, b].rearrange("c p d -> (c p) d"))
                    nc.sync.dma_start(out=tm[73:74, :], in_=ncp_out[b:b + 1, :])
                for dc in range(2):
                    pt = b0ps.tile([128, 128], F32, name="pt")
                    nc.tensor.transpose(pt, tm[:, ts(dc, 128)], ident_t)
                    nc.vector.tensor_copy(
                        hT[pair][:, dc, ds(half * 202 + t0, tn)], pt[:, 0:tn])

    # ================= PHASE B: layers =================
    # PSUM budget (8 banks): stat 1 + bc 2 + mm 2 + att 2 + den 1 = 8
    w_pool = tc.alloc_tile_pool(name="wts", bufs=2)
    y_pool = tc.alloc_tile_pool(name="y", bufs=4)
    qk_pool = tc.alloc_tile_pool(name="qk", bufs=2)
    v_pool = tc.alloc_tile_pool(name="v", bufs=3)
    e_pool = tc.alloc_tile_pool(name="e", bufs=8)
    o_pool = tc.alloc_tile_pool(name="o", bufs=2)
    a_pool = tc.alloc_tile_pool(name="a", bufs=2)
    u_pool = tc.alloc_tile_pool(name="u", bufs=3)
    r_pool = tc.alloc_tile_pool(name="r", bufs=1)
    d_pool = tc.alloc_tile_pool(name="d", bufs=2)
    ps_bc = tc.alloc_tile_pool(name="ps_bc", bufs=1, space="PSUM")
    ps_mm = tc.alloc_tile_pool(name="ps_mm", bufs=2, space="PSUM")
    ps_att = tc.alloc_tile_pool(name="ps_att", bufs=2, space="PSUM")
    ps_den = tc.alloc_tile_pool(name="ps_den", bufs=2, space="PSUM")

    def copy_any(k, out, in_):
        # alternate copies between ScalarE and VectorE to balance load
        if k % 2 == 0:
            nc.scalar.copy(out, in_)
        else:
            nc.vector.tensor_copy(out, in_)

    def layernorm(l, which):
        """LN over the dim axis of every pair's hT -> per-pair y16 bf16."""
        st_ps = ps_bc.tile([128, 2, 512], F32, name="bc")[0:64, 0, 0:404]
        nmm = 0
        for i in range(NPAIR):
            sq32 = u_pool.tile([128, 2, 404], F32R, name="sq32")
            for c in range(2):
                nc.vector.tensor_mul(sq32[:, c, :], hT[i][:, c, :], hT[i][:, c, :])
            for c in range(2):
                for row, srca in ((i, hT[i]), (32 + i, sq32)):
                    nc.tensor.matmul(
                        st_ps, colsel32_t[:, ds(63 - row, 64)], srca[:, c, :],
                        start=(nmm == 0), stop=(nmm == 63), skip_group_check=True)
                    nmm += 1
        mean = r_pool.tile([16, 404], F32, name="mean")
        nc.vector.tensor_scalar_mul(mean, st_ps[0:16, :], 1.0 / DIM)
        var = r_pool.tile([16, 404], F32, name="var")
        nc.vector.tensor_scalar(var, st_ps[32:48, :], 1.0 / DIM, EPS,
                                mybir.AluOpType.mult, mybir.AluOpType.add)
        msq = r_pool.tile([16, 404], F32, name="msq")
        nc.vector.tensor_mul(msq, mean, mean)
        nc.vector.tensor_sub(var, var, msq)
        lnv = r_pool.tile([16, 404], F32, name="lnv")
        nc.scalar.activation(lnv, var, AF.Ln)
        rstd = r_pool.tile([16, 404], F32, name="rstd")
        nc.scalar.activation(rstd, lnv, AF.Exp, scale=-0.5)
        ab = r_pool.tile([64, 404], BF16, name="ab")
        nc.vector.tensor_copy(ab[0:16, :], rstd)
        nc.vector.tensor_mul(ab[32:48, :], mean, rstd)
        return ab

    def ln_norm(i, ab, l, which):
        base = ((l * 2 + which) * 2) * 2
        bct = ps_bc.tile([128, 2, 512], F32, name="bc")
        rbc = bct[:, 0, 0:404]
        mbc = bct[:, 1, 0:404]
        nc.tensor.matmul(rbc, identb_t[0:16, ds(i, 1)].to_broadcast((16, 128)),
                         ab[0:16, :], start=True, stop=True, skip_group_check=True)
        nc.tensor.matmul(mbc, identb_t[32:48, ds(32 + i, 1)].to_broadcast((16, 128)),
                         ab[32:48, :], start=True, stop=True, skip_group_check=True)
        y16 = y_pool.tile([128, 2, 404], BF16, name="y16")
        for c in range(2):
            u = u_pool.tile([128, 404], F32, name="u")
            nc.vector.tensor_mul(u, hT[i][:, c, :], rbc)
            nc.vector.tensor_sub(u, u, mbc)
            nc.scalar.activation(
                y16[:, c, :], u, AF.Identity,
                bias=lnw_t[:, ds(base + 2 + c, 1)],
                scale=lnw_t[:, ds(base + c, 1)])
        return y16

    for l in range(DEPTH):
        wqkv_t = w_pool.tile([128, 2, 3 * INNER], BF16, name="wqkv")
        nc.sync.dma_start(
            out=wqkv_t,
            in_=wqkv.rearrange("p (l c f) -> p l c f", l=DEPTH, c=2)[:, l])
        wo_t = w_pool.tile([128, 4, DIM], BF16, name="wo")
        nc.sync.dma_start(
            out=wo_t, in_=wo.rearrange("p (l k f) -> p l k f", l=DEPTH, k=4)[:, l])
        wf1_t = w_pool.tile([128, 2, MLP], BF16, name="wf1")
        nc.sync.dma_start(
            out=wf1_t, in_=wf1.rearrange("p (l c f) -> p l c f", l=DEPTH, c=2)[:, l])
        wf2_t = w_pool.tile([128, 8, DIM], BF16, name="wf2")
        nc.sync.dma_start(
            out=wf2_t, in_=wf2.rearrange("p (l k f) -> p l k f", l=DEPTH, k=8)[:, l])
        ab1 = layernorm(l, 0)
        # ---- per pair: normalize -> QKV/V -> attention -> Wo+resid ----
        for i in range(NPAIR):
            y16 = ln_norm(i, ab1, l, 0)
            qk16 = qk_pool.tile([128, 8, 404], BF16, name="qk16")
            for fc in range(8):
                ps = ps_mm.tile([128, 512], F32, name="mm")
                for cc in range(2):
                    nc.tensor.matmul(
                        ps[:, 0:404], wqkv_t[:, cc, ts(fc, 128)], y16[:, cc, :],
                        start=(cc == 0), stop=(cc == 1), skip_group_check=True)
                copy_any(fc, qk16[:, fc, :], ps[:, 0:404])
            vpair = []
            for half in range(2):
                v16 = v_pool.tile([128, 2, INNER], BF16, name="v16")
                for tci, (t0, tn) in enumerate(TCH):
                    ps = ps_mm.tile([128, 512], F32, name="mm")
                    for cc in range(2):
                        nc.tensor.matmul(
                            ps[0:tn, :], y16[:, cc, ds(half * 202 + t0, tn)],
                            wqkv_t[:, cc, ds(2 * INNER, INNER)],
                            start=(cc == 0), stop=(cc == 1), skip_group_check=True)
                    copy_any(tci, v16[0:tn, tci, :], ps[0:tn, :])
                vpair.append(v16)
            # attention for both halves of the pair
            oT16 = o_pool.tile([128, 4, 404], BF16, name="oT16")
            for half in range(2):
                v16 = vpair[half]
                exps = []
                for h in range(8):
                    ro = (h % 2) * 64
                    e16 = e_pool.tile([128, 2, 202], BF16, name="e16")
                    for jci, (j0, jn) in enumerate(TCH):
                        sc = ps_att.tile([128, 202], F32, name="att")
                        nc.tensor.matmul(
                            sc[0:jn, :],
                            qk16[ds(ro, 64), 4 + h // 2, ds(half * 202 + j0, jn)],
                            qk16[ds(ro, 64), h // 2, ds(half * 202, 202)],
                            start=True, stop=True, skip_group_check=True)
                        nc.scalar.activation(e16[0:jn, jci, :], sc[0:jn, :],
                                             AF.Exp, scale=DH ** -0.5)
                    exps.append(e16)
                den = ps_den.tile([8, 202], F32, name="den")
                ndmm = 0
                for h in range(8):
                    for jci, (j0, jn) in enumerate(TCH):
                        nc.tensor.matmul(
                            den, colsel_t[0:jn, ds(63 - h, 8)],
                            exps[h][0:jn, jci, :],
                            start=(ndmm == 0), stop=(ndmm == 15),
                            skip_group_check=True)
                        ndmm += 1
                lnd = d_pool.tile([8, 202], F32, name="lnd")
                nc.scalar.activation(lnd, den, AF.Ln)
                recip8 = d_pool.tile([8, 202], BF16, name="recip8")
                nc.scalar.activation(recip8, lnd, AF.Exp, scale=-1.0)
                for c in range(4):
                    po = ps_att.tile([128, 202], F32, name="att")
                    for hh in range(2):
                        h = 2 * c + hh
                        for jci, (j0, jn) in enumerate(TCH):
                            nc.tensor.matmul(
                                po[ds(hh * 64, 64), :],
                                v16[0:jn, jci, ds(h * DH, DH)],
                                exps[h][0:jn, jci, :],
                                start=(jci == 0), stop=(jci == 1),
                                skip_group_check=True)
                    rbc = ps_mm.tile([128, 512], F32, name="mm")
                    nc.tensor.matmul(rbc[:, 0:202], esel_t[:, c, :], recip8,
                                     start=True, stop=True, skip_group_check=True)
                    o32 = u_pool.tile([128, 202], F32, name="o32")
                    copy_any(c, o32, po)
                    nc.vector.tensor_mul(oT16[:, c, ds(half * 202, 202)],
                                         o32, rbc[:, 0:202])
            # Wo + bo + residual
            for fc in range(2):
                ps = ps_mm.tile([128, 512], F32, name="mm")
                for kc in range(4):
                    nc.tensor.matmul(
                        ps[:, 0:404], wo_t[:, kc, ts(fc, 128)], oT16[:, kc, :],
                        start=(kc == 0), stop=False, skip_group_check=True)
                nc.tensor.matmul(
                    ps[:, 0:404], bo_t[0:1, ds(l * DIM + fc * 128, 128)], onesn_t,
                    start=False, stop=True, skip_group_check=True)
                nc.vector.tensor_add(hT[i][:, fc, :], hT[i][:, fc, :], ps[:, 0:404])
        # ---- LN2 + FF (per pair) ----
        ab2 = layernorm(l, 1)
        for i in range(NPAIR):
            y2 = ln_norm(i, ab2, l, 1)
            aT16 = a_pool.tile([128, 8, 404], BF16, name="aT16")
            for mc in range(8):
                ps = ps_mm.tile([128, 512], F32, name="mm")
                for cc in range(2):
                    nc.tensor.matmul(
                        ps[:, 0:404], wf1_t[:, cc, ts(mc, 128)], y2[:, cc, :],
                        start=(cc == 0), stop=False, skip_group_check=True)
                nc.tensor.matmul(
                    ps[:, 0:404], bf1_t[0:1, ds(l * MLP + mc * 128, 128)], onesn_t,
                    start=False, stop=True, skip_group_check=True)
                nc.scalar.activation(aT16[:, mc, :], ps[:, 0:404], AF.Gelu)
            for fc in range(2):
                ps = ps_mm.tile([128, 512], F32, name="mm")
                for kc in range(8):
                    nc.tensor.matmul(
                        ps[:, 0:404], wf2_t[:, kc, ts(fc, 128)], aT16[:, kc, :],
                        start=(kc == 0), stop=False, skip_group_check=True)
                nc.tensor.matmul(
                    ps[:, 0:404], bf2_t[0:1, ds(l * DIM + fc * 128, 128)], onesn_t,
                    start=False, stop=True, skip_group_check=True)
                nc.vector.tensor_add(hT[i][:, fc, :], hT[i][:, fc, :], ps[:, 0:404])

    # ================= HEAD =================
    with tc.tile_pool(name="hd", bufs=1) as hd:
        clsT = hd.tile([128, 2, BL], F32)
        for b in range(BL):
            pair, half = b // 2, b % 2
            for c in range(2):
                copy_any(b + c, clsT[:, c, ds(b, 1)],
                         hT[pair][:, c, ds(half * 202, 1)])
        cls_tm = hd.tile([32, 2, 128], F32)
        for c in range(2):
            pt = ps_mm.tile([128, 512], F32, name="mm")
            nc.tensor.transpose(pt[0:BL, 0:128], clsT[:, c, :], ident_t)
            nc.vector.tensor_copy(cls_tm[:, c, :], pt[0:BL, 0:128])
        s1 = hd.tile([32, 1], F32)
        nc.vector.reduce_sum(s1, cls_tm, axis=mybir.AxisListType.XY)
        sqt = hd.tile([32, 2, 128], F32)
        nc.vector.tensor_mul(sqt, cls_tm, cls_tm)
        s2 = hd.tile([32, 1], F32)
        nc.vector.reduce_sum(s2, sqt, axis=mybir.AxisListType.XY)
        mean = hd.tile([32, 1], F32)
        nc.vector.tensor_scalar_mul(mean, s1, 1.0 / DIM)
        var = hd.tile([32, 1], F32)
        nc.vector.tensor_scalar(var, s2, 1.0 / DIM, EPS,
                                mybir.AluOpType.mult, mybir.AluOpType.add)
        msq = hd.tile([32, 1], F32)
        nc.vector.tensor_mul(msq, mean, mean)
        nc.vector.tensor_sub(var, var, msq)
        lnv = hd.tile([32, 1], F32)
        nc.scalar.activation(lnv, var, AF.Ln)
        rstd = hd.tile([32, 1], F32)
        nc.scalar.activation(rstd, lnv, AF.Exp, scale=-0.5)
        z = hd.tile([32, 2, 128], F32)
        nc.vector.tensor_scalar(z, cls_tm, mean, rstd,
                                mybir.AluOpType.subtract, mybir.AluOpType.mult)
        gbt = ps_bc.tile([128, 2, 512], F32, name="bc")
        gb = gbt[:, 0, 0:404]
        bb = gbt[:, 1, 0:404]
        nc.tensor.matmul(gb[0:BL, 0:256], identb_t[0:2, ds(0, 1)].to_broadcast((2, BL)),
                         lnhrow_t, start=True, stop=True, skip_group_check=True)
        nc.tensor.matmul(bb[0:BL, 0:256], identb_t[0:2, ds(1, 1)].to_broadcast((2, BL)),
                         lnhrow_t, start=True, stop=True, skip_group_check=True)
        zf = hd.tile([32, 256], F32)
        nc.vector.tensor_mul(zf, z.rearrange("p a b -> p (a b)"), gb[0:BL, 0:256])
        nc.vector.tensor_add(zf, zf, bb[0:BL, 0:256])
        zT = hd.tile([128, 2, BL], BF16)
        for c in range(2):
            pt = ps_mm.tile([128, 512], F32, name="mm")
            nc.tensor.transpose(pt[0:128, 0:BL], zf[:, ts(c, 128)],
                                ident_t[0:BL, 0:BL])
            nc.vector.tensor_copy(zT[:, c, :], pt[0:128, 0:BL])
        r1 = hd.tile([32, 1024], F32)
        for nh in range(2):
            ps = ps_mm.tile([128, 512], F32, name="mm")
            for cc in range(2):
                nc.tensor.matmul(ps[0:BL, :], zT[:, cc, :],
                                 wh1_t[:, cc, ts(nh, 512)],
                                 start=(cc == 0), stop=False, skip_group_check=True)
            nc.tensor.matmul(ps[0:BL, :], ones1_t[0:1, 0:BL], bh1_t[0:1, ts(nh, 512)],
                             start=False, stop=True, skip_group_check=True)
            nc.scalar.activation(r1[:, ts(nh, 512)], ps[0:BL, :], AF.Relu)
        r1T = hd.tile([128, 8, BL], BF16)
        for kc in range(8):
            pt = ps_mm.tile([128, 512], F32, name="mm")
            nc.tensor.transpose(pt[0:128, 0:BL], r1[:, ts(kc, 128)],
                                ident_t[0:BL, 0:BL])
            nc.vector.tensor_copy(r1T[:, kc, :], pt[0:128, 0:BL])
        po = ps_mm.tile([128, 512], F32, name="mm")
        for kc in range(8):
            nc.tensor.matmul(po[0:OUT, 0:BL], wh2_t[:, kc, :], r1T[:, kc, :],
                             start=(kc == 0), stop=(kc == 7), skip_group_check=True)
        outs = hd.tile([OUT, BL], F32)
        nc.scalar.add(outs, po[0:OUT, 0:BL], bh2_t)
        nc.sync.dma_start(out=out_d, in_=outs)

    for pl_ in (ps_den, ps_att, ps_mm, ps_bc, d_pool, r_pool, u_pool,
                a_pool, o_pool, e_pool, v_pool, qk_pool, y_pool, w_pool, dram,
                singles):
        pl_.release()
    tcx.__exit__(None, None, None)

    # workaround: this walrus rejects >1 sync-wait per instruction
    from wsplit import split_excess_waits
    split_excess_waits(nc)
    return nc


# --------------------------------------------------------------------------
# host side
# --------------------------------------------------------------------------
def _prep_inputs(inputs):
    bf = ml_dtypes.bfloat16
    x = np.asarray(inputs["x"], np.float32)
    gene_idx = np.asarray(inputs["gene_idx"]).astype(np.int64)
    nc_idx = np.asarray(inputs["nc_idx"]).astype(np.int64)
    W_path = np.asarray(inputs["W_path"], np.float32)
    b_path = np.asarray(inputs["b_path"], np.float32)
    W_nc = np.asarray(inputs["W_nc"], np.float32)
    b_nc = np.asarray(inputs["b_nc"], np.float32)
    cls_token = np.asarray(inputs["cls_token"], np.float32).reshape(1, DIM)
    Wqkv = np.asarray(inputs["Wqkv"], np.float32)
    Wo = np.asarray(inputs["Wo"], np.float32)
    bo = np.asarray(inputs["bo"], np.float32)
    ln1_g = np.asarray(inputs["ln1_g"], np.float32)
    ln1_b = np.asarray(inputs["ln1_b"], np.float32)
    ln2_g = np.asarray(inputs["ln2_g"], np.float32)
    ln2_b = np.asarray(inputs["ln2_b"], np.float32)
    Wff1 = np.asarray(inputs["Wff1"], np.float32)
    bff1 = np.asarray(inputs["bff1"], np.float32)
    Wff2 = np.asarray(inputs["Wff2"], np.float32)
    bff2 = np.asarray(inputs["bff2"], np.float32)
    lnh_g = np.asarray(inputs["lnh_g"], np.float32)
    lnh_b = np.asarray(inputs["lnh_b"], np.float32)
    Wh1 = np.asarray(inputs["Wh1"], np.float32)
    bh1 = np.asarray(inputs["bh1"], np.float32)
    Wh2 = np.asarray(inputs["Wh2"], np.float32)
    bh2 = np.asarray(inputs["bh2"], np.float32)

    xr = x.reshape(B, G * O)
    rows_all = ((gene_idx * O)[:, :, None] + np.arange(O)[None, None, :]).reshape(
        P, MAXG * O)
    nc_rows = ((nc_idx * O)[:, None] + np.arange(O)[None, :]).reshape(NCTOK * O)

    # shared (identical on every core) prepped weights
    shared = {}
    shared["cls32"] = cls_token
    shared["wqkv"] = np.ascontiguousarray(
        Wqkv.reshape(DEPTH, 2, 128, 3 * INNER).transpose(2, 0, 1, 3)
    ).astype(bf).reshape(128, DEPTH * 2 * 3 * INNER)
    shared["wo"] = np.ascontiguousarray(
        Wo.reshape(DEPTH, 4, 128, DIM).transpose(2, 0, 1, 3)
    ).astype(bf).reshape(128, DEPTH * 4 * DIM)
    shared["wf1"] = np.ascontiguousarray(
        Wff1.reshape(DEPTH, 2, 128, MLP).transpose(2, 0, 1, 3)
    ).astype(bf).reshape(128, DEPTH * 2 * MLP)
    shared["wf2"] = np.ascontiguousarray(
        Wff2.reshape(DEPTH, 8, 128, DIM).transpose(2, 0, 1, 3)
    ).astype(bf).reshape(128, DEPTH * 8 * DIM)
    shared["bo16"] = bo.astype(bf).reshape(1, DEPTH * DIM)
    shared["bf1_16"] = bff1.astype(bf).reshape(1, DEPTH * MLP)
    shared["bf2_16"] = bff2.astype(bf).reshape(1, DEPTH * DIM)
    lnwa = np.zeros((128, 36), np.float32)
    for l in range(DEPTH):
        for which, (gv, bv) in enumerate(((ln1_g[l], ln1_b[l]),
                                          (ln2_g[l], ln2_b[l]))):
            base = ((l * 2 + which) * 2) * 2
            for cc in range(2):
                lnwa[:, base + cc] = gv[cc * 128:(cc + 1) * 128]
                lnwa[:, base + 2 + cc] = bv[cc * 128:(cc + 1) * 128]
    lnwa[:, 32] = lnh_g[0:128]
    lnwa[:, 33] = lnh_g[128:256]
    lnwa[:, 34] = lnh_b[0:128]
    lnwa[:, 35] = lnh_b[128:256]
    shared["lnw"] = lnwa
    shared["lnhrow"] = np.stack([lnh_g, lnh_b]).astype(bf)
    csel = np.zeros((128, 127), np.float32)
    csel[:, 63] = 1.0
    shared["colsel"] = csel.astype(bf)
    shared["colsel32"] = csel
    shared["ones1"] = np.ones((1, 128), dtype=bf)
    shared["onesn"] = np.ones((1, 404), dtype=bf)
    es = np.zeros((8, 4, 128), np.float32)
    for cc in range(4):
        es[2 * cc, cc, 0:64] = 1.0
        es[2 * cc + 1, cc, 64:128] = 1.0
    shared["esel"] = es.astype(bf).reshape(8, 4 * 128)
    shared["ident"] = np.eye(128, dtype=np.float32)
    shared["identb"] = np.eye(128, dtype=np.float32).astype(bf)
    shared["wh1"] = np.ascontiguousarray(
        Wh1.reshape(2, 128, 4 * DIM).transpose(1, 0, 2)
    ).astype(bf).reshape(128, 2 * 4 * DIM)
    shared["wh2"] = np.ascontiguousarray(
        Wh2.reshape(8, 128, OUT).transpose(1, 0, 2)
    ).astype(bf).reshape(128, 8 * OUT)
    shared["bh1"] = bh1.astype(bf).reshape(1, 4 * DIM)
    shared["bh2c"] = bh2.astype(np.float32).reshape(OUT, 1)

    per_core = []
    for c in range(NCORES):
        d = dict(shared)
        pslice = slice(c * PL, (c + 1) * PL)
        gTc = np.zeros((PL, KPATHP, B), dtype=bf)
        Wpc = np.zeros((PL, KPATHP, DIM), dtype=bf)
        for j, p in enumerate(range(c * PL, (c + 1) * PL)):
            gTc[j, :MAXG * O] = xr[:, rows_all[p]].T.astype(bf)
            Wpc[j, :MAXG * O] = W_path[p].astype(bf)
        d["gT"] = gTc
        d["wp"] = Wpc
        d["bp"] = b_path[pslice].astype(bf).reshape(1, PL * DIM)
        ks = slice(c * KNC, (c + 1) * KNC)
        ncgc = np.zeros((KNCP, B), dtype=bf)
        ncgc[:KNC] = xr[:, nc_rows[ks]].T.astype(bf)
        wncc = np.zeros((KNCP, DIM), dtype=bf)
        wncc[:KNC] = W_nc[ks].astype(bf)
        d["ncg"] = ncgc
        d["wnc"] = wncc
        # ReduceScatter sums 8 partials; only core 0 contributes b_nc.
        d["bnc"] = (b_nc if c == 0 else np.zeros_like(b_nc)).astype(bf).reshape(1, DIM)
        per_core.append(d)
    return per_core


def kernel(**inputs):
    if "nc" not in _CACHE:
        _CACHE["nc"] = _build()
    nc = _CACHE["nc"]
    per_core = _prep_inputs(inputs)
    r = run_bass_kernel_spmd(nc, per_core, core_ids=list(range(NCORES)))
    outs = [np.asarray(r.results[c]["out"]).T for c in range(NCORES)]
    return np.concatenate(outs, axis=0).astype(np.float32)
